# revision 6
# baseline (speedup 1.0000x reference)
"""Trainium2 Bass kernel for the 21-joint hand-graph message-passing MLP.

Math (per sample b, per target joint t with neighbor list S_t of length n):
    g   = concat(x[b, S_t[0]], ..., x[b, S_t[n-1]])          # [n*64]
    h1  = relu(g @ W1_t + b1_t)                              # [128]
    h2  = relu(h1 @ W2_t + b2_t)                             # [128]
    out[b, t] = h2 @ W3_t + b3_t                             # [64]

Strategy (pure data parallel over 8 NeuronCores, B=65536 -> 8192/core):
  - x is read DIRECTLY from its fp32 DRAM layout by bf16-bitcast xbar
    DMA-transposes: the host zeroes the low 16 bits of every fp32 (pure
    truncation, values unchanged within bf16 precision), so the bf16
    bitcast view of node j's 64 features is 128 contiguous bf16 columns
    whose odd columns are the truncated-bf16 features and even columns
    are +0.0.  One [TILE,128] transpose per node per batch tile produces
    xN[j] = [128 partitions (feature f at partition 2f+1, zeros at even
    partitions), TILE batch].  No fp32->bf16 cast pre-pass, no DRAM
    round trip: per-core HBM traffic is 43MB read + 21.5MB write.
  - L1 runs weight-stationary with one K=128 chunk per (target, neighbor)
    instance (69 chunks); W1 rows are interleaved to match (odd rows =
    weights, even rows = 0 hitting the +0.0 partitions).
  - L1/L2 relu+bias are fused into the PSUM->SBUF evacuation, alternated
    between ScalarE and VectorE (the only PSUM readers).
  - L3 is W3-stationary with the output FEATURE-major: two targets share
    one PSUM bank via column tiling (psum[0:64]=target a, psum[64:128]=b,
    concurrent M=64 matmuls), N=512 per matmul.  b3 is added during
    evacuation (per-partition bias).  The store is a plain bf16 DMA to
    out[1344, BC]; the host does the final [B,21,64] transpose + fp32
    cast.  L3 is interleaved after each pair of targets so only a few h2
    tiles are live.
  - Transposes are split between the two HWDGE issuers (Sync + ScalarE)
    to halve per-sequencer descriptor-generation occupancy; output
    stores ride the otherwise-idle GpSimd SWDGE path.
"""

import os
import numpy as np
import ml_dtypes

B, J, D, H1, H2 = 65536, 21, 64, 128, 128
NCORES = 8
BC = B // NCORES          # 8192 samples per core
TILE = 1024               # batch tile (2 PSUM banks wide in fp32)
NTILES = BC // TILE       # 8

FINGER_BASE = [4 * f + 1 for f in range(5)]
NEIGH = {
    6: [[0, 1, 5, 9, 13, 17]],
    5: [[0, 5, 6, 1, 9], [0, 9, 10, 5, 13], [0, 13, 14, 9, 17]],
    4: [[0, 1, 2, 5], [0, 17, 18, 13]],
    3: [r for b in FINGER_BASE for r in ([b, b + 1, b + 2], [b + 1, b + 2, b + 3])],
    2: [[b + 2, b + 3] for b in FINGER_BASE],
}
OUT = {
    6: [0],
    5: [5, 9, 13],
    4: [1, 17],
    3: [j for b in FINGER_BASE for j in (b + 1, b + 2)],
    2: [b + 3 for b in FINGER_BASE],
}
GROUPS = [6, 5, 4, 3, 2]

# target t -> (n, row index within its group, neighbor list)
TARGET = {}
for n in GROUPS:
    for row, t in enumerate(OUT[n]):
        TARGET[t] = (n, row, list(NEIGH[n][row]))

TOTAL_CHUNKS = sum(TARGET[t][0] for t in range(21))   # 69: one per neighbor

# deterministic column layout of packed W1 chunks: order by (t, pos)
CHUNK_COLS = {}
_col = 0
for _t in range(21):
    for _ci in range(TARGET[_t][0]):
        CHUNK_COLS[(_t, _ci)] = _col
        _col += 128

# L3 pairs of targets sharing one PSUM bank via column tiling
L3_PAIRS = [(2 * i, 2 * i + 1) for i in range(10)] + [(20,)]


def pack_weights(inputs):
    """Host-side prep: permute/pack all weights into a handful of flat arrays.

    W1 chunks are row-interleaved: packed row 2r+1 = W1 row 64*pos + r,
    packed row 2r = 0 (multiplies the +0.0 at even partitions of xN).
    """
    bf16 = ml_dtypes.bfloat16
    w1p = np.zeros((128, 128 * TOTAL_CHUNKS), np.float32)
    for t in range(21):
        n, row, S = TARGET[t]
        W1 = np.asarray(inputs[f"w1_g{n}"][row], np.float32)  # [n*64, 128]
        for ci in range(n):
            col = CHUNK_COLS[(t, ci)]
            w1p[1::2, col:col + 128] = W1[64 * ci:64 * ci + 64]
    w2p = np.zeros((128, 128 * 21), np.float32)
    w3p = np.zeros((128, 64 * 21), np.float32)
    b1p = np.zeros((128, 21), np.float32)
    b2p = np.zeros((128, 21), np.float32)
    b3p = np.zeros((128, len(L3_PAIRS)), np.float32)
    for t in range(21):
        n, row, _ = TARGET[t]
        w2p[:, 128 * t:128 * (t + 1)] = np.asarray(inputs[f"w2_g{n}"][row])
        w3p[:, 64 * t:64 * (t + 1)] = np.asarray(inputs[f"w3_g{n}"][row])
        b1p[:, t] = np.asarray(inputs[f"b1_g{n}"][row])
        b2p[:, t] = np.asarray(inputs[f"b2_g{n}"][row])
    for pi, pr in enumerate(L3_PAIRS):
        for k, t in enumerate(pr):
            n, row, _ = TARGET[t]
            b3p[64 * k:64 * k + 64, pi] = np.asarray(inputs[f"b3_g{n}"][row])
    return dict(
        w1p=w1p.astype(bf16), w2p=w2p.astype(bf16), w3p=w3p.astype(bf16),
        b1p=b1p, b2p=b2p, b3p=b3p,
    )


def numpy_emulate(inputs, x):
    """Bit-layout-faithful numpy model of what the HW kernel computes (minus
    PSUM rounding): used to validate the plan / packing offline."""
    bf16 = ml_dtypes.bfloat16
    packed = pack_weights(inputs)
    # truncate-to-bf16 (high 16 bits of fp32), as the bitcast transpose sees
    xb = (np.ascontiguousarray(x, np.float32).view(np.uint32) >> 16).astype(
        np.uint16).view(bf16)                            # [Bn, 21, 64]
    Bn = x.shape[0]
    out = np.zeros((Bn, 21, 64), np.float32)
    for t in range(21):
        n, row, S = TARGET[t]
        psum1 = np.zeros((128, Bn), np.float32)
        for ci in range(n):
            col = CHUNK_COLS[(t, ci)]
            lhsT = packed["w1p"][:, col:col + 128].astype(np.float32)
            rhs = np.zeros((128, Bn), np.float32)
            rhs[1::2] = xb[:, S[ci]].astype(np.float32).T
            psum1 += lhsT.T @ rhs
        h1 = np.maximum(psum1 + packed["b1p"][:, t:t + 1], 0).astype(bf16)
        w2 = packed["w2p"][:, 128 * t:128 * (t + 1)].astype(np.float32)
        psum2 = w2.T @ h1.astype(np.float32)
        h2 = np.maximum(psum2 + packed["b2p"][:, t:t + 1], 0).astype(bf16)
        w3 = packed["w3p"][:, 64 * t:64 * (t + 1)].astype(np.float32)
        psum3 = w3.T @ h2.astype(np.float32)             # [64, Bn]
        pi, k = (t // 2, t % 2) if t < 20 else (10, 0)
        b3 = packed["b3p"][64 * k:64 * k + 64, pi]
        out[:, t] = (psum3 + b3[:, None]).astype(bf16).astype(np.float32).T
    return out


# ---------------------------------------------------------------------------
# Bass kernel
# ---------------------------------------------------------------------------

def build_bass_kernel():
    import concourse.bass as bass
    import concourse.tile as tile
    from concourse import bacc, mybir

    bf16 = mybir.dt.bfloat16
    f32 = mybir.dt.float32
    Relu = mybir.ActivationFunctionType.Relu
    Ident = mybir.ActivationFunctionType.Identity
    Alu = mybir.AluOpType

    nc = bacc.Bacc("TRN2", target_bir_lowering=False, debug=False,
                   num_devices=NCORES)
    x_dram = nc.dram_tensor("x", [BC, J, D], f32, kind="ExternalInput").ap()
    out_dram = nc.dram_tensor("out", [J * D, BC], bf16, kind="ExternalOutput").ap()
    w1_dram = nc.dram_tensor("w1p", [128, 128 * TOTAL_CHUNKS], bf16,
                             kind="ExternalInput").ap()
    w2_dram = nc.dram_tensor("w2p", [128, 128 * 21], bf16, kind="ExternalInput").ap()
    w3_dram = nc.dram_tensor("w3p", [128, 64 * 21], bf16, kind="ExternalInput").ap()
    b1_dram = nc.dram_tensor("b1p", [128, 21], f32, kind="ExternalInput").ap()
    b2_dram = nc.dram_tensor("b2p", [128, 21], f32, kind="ExternalInput").ap()
    b3_dram = nc.dram_tensor("b3p", [128, len(L3_PAIRS)], f32,
                             kind="ExternalInput").ap()

    # bf16 bitcast view of x: [BC, J, 128]; node j's block is contiguous,
    # odd columns are truncated-bf16 features, even columns +0.0
    x_bits = x_dram.bitcast(bf16)

    with tile.TileContext(nc) as tc:
        with (
            tc.tile_pool(name="wpool", bufs=1) as wpool,
            tc.tile_pool(name="xtp", bufs=2) as xtp,
            tc.tile_pool(name="actp", bufs=2) as actp,
            tc.tile_pool(name="h2p", bufs=2) as h2p,
            tc.tile_pool(name="stgp", bufs=2) as stgp,
            tc.tile_pool(name="ps1", bufs=2, space="PSUM") as ps1,
            tc.tile_pool(name="ps2", bufs=1, space="PSUM") as ps2,
            tc.tile_pool(name="ps3", bufs=2, space="PSUM") as ps3,
        ):
            w1s = wpool.tile([128, 128 * TOTAL_CHUNKS], bf16, name="w1s")
            w2s = wpool.tile([128, 128 * 21], bf16, name="w2s")
            w3s = wpool.tile([128, 64 * 21], bf16, name="w3s")
            b1s = wpool.tile([128, 21], f32, name="b1s")
            b2s = wpool.tile([128, 21], f32, name="b2s")
            b3s = wpool.tile([128, len(L3_PAIRS)], f32, name="b3s")
            nc.sync.dma_start(w1s[:], w1_dram)
            nc.sync.dma_start(w2s[:], w2_dram)
            nc.sync.dma_start(w3s[:], w3_dram)
            nc.sync.dma_start(b1s[:], b1_dram)
            nc.sync.dma_start(b2s[:], b2_dram)
            nc.sync.dma_start(b3s[:], b3_dram)

            # evac engine round-robin between the two PSUM readers
            evac_state = [0]

            def evac(dst, src, bias, relu):
                evac_state[0] ^= 1
                if evac_state[0]:
                    nc.scalar.activation(dst, src, Relu if relu else Ident,
                                         bias=bias, scale=1.0)
                else:
                    if relu:
                        nc.vector.tensor_scalar(dst, src, bias, 0.0,
                                                Alu.add, Alu.max)
                    else:
                        nc.vector.tensor_scalar(dst, src, bias, None, Alu.add)

            for it in range(NTILES):
                b0 = it * TILE
                # ---- per-node feature-major tiles via bitcast transpose ----
                xN = []
                for j in range(J):
                    xt = xtp.tile([128, TILE], bf16, tag=f"xn{j}", name=f"xn{j}")
                    # all transposes on ONE HWDGE queue: concurrent xbar
                    # transposes from two rings corrupt each other
                    nc.sync.dma_start(xt[:], x_bits[b0:b0 + TILE, j, :],
                                      transpose=True)
                    xN.append(xt)

                def mlp12(t):
                    n, _, S = TARGET[t]
                    psum1 = ps1.tile([128, TILE], f32, tag="psum1", name="psum1")
                    for h in range(TILE // 512):
                        for ci in range(n):
                            col = CHUNK_COLS[(t, ci)]
                            nc.tensor.matmul(
                                psum1[:, 512 * h:512 * (h + 1)],
                                w1s[:, col:col + 128],
                                xN[S[ci]][:, 512 * h:512 * (h + 1)],
                                start=(ci == 0), stop=(ci == n - 1))
                    h1 = actp.tile([128, TILE], bf16, tag="h1", name="h1")
                    evac(h1[:], psum1[:], b1s[:, t:t + 1], relu=True)

                    psum2 = ps2.tile([128, TILE], f32, tag="psum2", name="psum2")
                    for h in range(TILE // 512):
                        nc.tensor.matmul(
                            psum2[:, 512 * h:512 * (h + 1)],
                            w2s[:, 128 * t:128 * (t + 1)],
                            h1[:, 512 * h:512 * (h + 1)],
                            start=True, stop=True)
                    h2 = h2p.tile([128, TILE], bf16, tag=f"h2_{t % 4}",
                                  name=f"h2_{t % 4}")
                    evac(h2[:], psum2[:], b2s[:, t:t + 1], relu=True)
                    return h2

                # ---- fused L1/L2 then L3 per pair of targets ----
                for pi, pr in enumerate(L3_PAIRS):
                    h2t = [mlp12(t) for t in pr]
                    m = 64 * len(pr)
                    stg = stgp.tile([m, TILE], bf16, tag=f"stg{pi}",
                                    name=f"stg{pi}")
                    for h in range(TILE // 512):
                        psum3 = ps3.tile([m, 512], f32, tag="psum3", name="psum3")
                        for k, t in enumerate(pr):
                            nc.tensor.matmul(
                                psum3[64 * k:64 * (k + 1), :],
                                w3s[:, 64 * t:64 * (t + 1)],
                                h2t[k][:, 512 * h:512 * (h + 1)],
                                start=True, stop=True,
                                skip_group_check=True)
                        evac(stg[:, 512 * h:512 * (h + 1)], psum3[:],
                             b3s[0:m, pi:pi + 1], relu=False)
                    nc.gpsimd.dma_start(
                        out_dram[128 * pi:128 * pi + m, b0:b0 + TILE], stg[:])

    nc.compile()
    return nc


PACKED = None
_NC = None
LAST_RESULT = None


def prepare(inputs):
    """Build (once) the bass module and the per-core input maps."""
    global PACKED, _NC
    import sys
    if "/opt/trn_rl_repo" not in sys.path:
        sys.path.insert(0, "/opt/trn_rl_repo")
    x = np.ascontiguousarray(np.asarray(inputs["x"], np.float32))
    # zero the low 16 bits of every fp32 (truncate to bf16 precision) so the
    # even bf16 columns of the bitcast view are +0.0, never NaN/Inf garbage
    x = (x.view(np.uint32) & np.uint32(0xFFFF0000)).view(np.float32)
    PACKED = pack_weights(inputs)
    if _NC is None:
        _NC = build_bass_kernel()
    in_maps = []
    for core in range(NCORES):
        m = dict(PACKED)
        m["x"] = x[core * BC:(core + 1) * BC]
        in_maps.append(m)
    return _NC, in_maps


def kernel(**inputs):
    global LAST_RESULT
    nc, in_maps = prepare(inputs)
    from concourse.bass_utils import run_bass_kernel_spmd
    res = run_bass_kernel_spmd(nc, in_maps, core_ids=list(range(NCORES)))
    LAST_RESULT = res
    # per-core out: [21*64, BC] bf16, feature-major
    full = np.concatenate([r["out"] for r in res.results], 1)  # [1344, B]
    return np.ascontiguousarray(
        full.reshape(J, D, B).transpose(2, 0, 1)).astype(np.float32)


# revision 7
# speedup vs baseline: 1.3376x; 1.3376x over previous
"""Trainium2 Bass kernel for the 21-joint hand-graph message-passing MLP.

Math (per sample b, per target joint t with neighbor list S_t of length n):
    g   = concat(x[b, S_t[0]], ..., x[b, S_t[n-1]])          # [n*64]
    h1  = relu(g @ W1_t + b1_t)                              # [128]
    h2  = relu(h1 @ W2_t + b2_t)                             # [128]
    out[b, t] = h2 @ W3_t + b3_t                             # [64]

Strategy (pure data parallel over 8 NeuronCores, B=65536 -> 8192/core):
  - x is read DIRECTLY from its fp32 DRAM layout by bf16-bitcast xbar
    DMA-transposes: the host zeroes the low 16 bits of every fp32 (pure
    truncation, values unchanged within bf16 precision), so the bf16
    bitcast view of node j's 64 features is 128 contiguous bf16 columns
    whose odd columns are the truncated-bf16 features and even columns
    are +0.0.  One [TILE,128] transpose per node per batch tile produces
    xN[j] = [128 partitions (feature f at partition 2f+1, zeros at even
    partitions), TILE batch].  No fp32->bf16 cast pre-pass, no DRAM
    round trip: per-core HBM traffic is 43MB read + 21.5MB write.
  - L1 runs weight-stationary with one K=128 chunk per (target, neighbor)
    instance (69 chunks); W1 rows are interleaved to match (odd rows =
    weights, even rows = 0 hitting the +0.0 partitions).
  - L1/L2 relu+bias are fused into the PSUM->SBUF evacuation, alternated
    between ScalarE and VectorE (the only PSUM readers).
  - L3 is W3-stationary with the output FEATURE-major: two targets share
    one PSUM bank via column tiling (psum[0:64]=target a, psum[64:128]=b,
    concurrent M=64 matmuls), N=512 per matmul.  b3 is added during
    evacuation (per-partition bias).  The store is a plain bf16 DMA to
    out[1344, BC]; the host does the final [B,21,64] transpose + fp32
    cast.  L3 is interleaved after each pair of targets so only a few h2
    tiles are live.
  - Transposes are split between the two HWDGE issuers (Sync + ScalarE)
    to halve per-sequencer descriptor-generation occupancy; output
    stores ride the otherwise-idle GpSimd SWDGE path.
"""

import os
import numpy as np
import ml_dtypes

B, J, D, H1, H2 = 65536, 21, 64, 128, 128
NCORES = 8
BC = B // NCORES          # 8192 samples per core
TILE = 1024               # batch tile (2 PSUM banks wide in fp32)
NTILES = BC // TILE       # 8

FINGER_BASE = [4 * f + 1 for f in range(5)]
NEIGH = {
    6: [[0, 1, 5, 9, 13, 17]],
    5: [[0, 5, 6, 1, 9], [0, 9, 10, 5, 13], [0, 13, 14, 9, 17]],
    4: [[0, 1, 2, 5], [0, 17, 18, 13]],
    3: [r for b in FINGER_BASE for r in ([b, b + 1, b + 2], [b + 1, b + 2, b + 3])],
    2: [[b + 2, b + 3] for b in FINGER_BASE],
}
OUT = {
    6: [0],
    5: [5, 9, 13],
    4: [1, 17],
    3: [j for b in FINGER_BASE for j in (b + 1, b + 2)],
    2: [b + 3 for b in FINGER_BASE],
}
GROUPS = [6, 5, 4, 3, 2]

# target t -> (n, row index within its group, neighbor list)
TARGET = {}
for n in GROUPS:
    for row, t in enumerate(OUT[n]):
        TARGET[t] = (n, row, list(NEIGH[n][row]))

TOTAL_CHUNKS = sum(TARGET[t][0] for t in range(21))   # 69: one per neighbor

# deterministic column layout of packed W1 chunks: order by (t, pos)
CHUNK_COLS = {}
_col = 0
for _t in range(21):
    for _ci in range(TARGET[_t][0]):
        CHUNK_COLS[(_t, _ci)] = _col
        _col += 128

# L3 pairs of targets sharing one PSUM bank via column tiling
L3_PAIRS = [(2 * i, 2 * i + 1) for i in range(10)] + [(20,)]


def pack_weights(inputs):
    """Host-side prep: permute/pack all weights into a handful of flat arrays.

    W1 chunks are row-interleaved: packed row 2r+1 = W1 row 64*pos + r,
    packed row 2r = 0 (multiplies the +0.0 at even partitions of xN).
    """
    bf16 = ml_dtypes.bfloat16
    w1p = np.zeros((128, 128 * TOTAL_CHUNKS), np.float32)
    for t in range(21):
        n, row, S = TARGET[t]
        W1 = np.asarray(inputs[f"w1_g{n}"][row], np.float32)  # [n*64, 128]
        for ci in range(n):
            col = CHUNK_COLS[(t, ci)]
            w1p[1::2, col:col + 128] = W1[64 * ci:64 * ci + 64]
    w2p = np.zeros((128, 128 * 21), np.float32)
    w3p = np.zeros((128, 64 * 21), np.float32)
    b1p = np.zeros((128, 21), np.float32)
    b2p = np.zeros((128, 21), np.float32)
    b3p = np.zeros((128, len(L3_PAIRS)), np.float32)
    for t in range(21):
        n, row, _ = TARGET[t]
        w2p[:, 128 * t:128 * (t + 1)] = np.asarray(inputs[f"w2_g{n}"][row])
        w3p[:, 64 * t:64 * (t + 1)] = np.asarray(inputs[f"w3_g{n}"][row])
        b1p[:, t] = np.asarray(inputs[f"b1_g{n}"][row])
        b2p[:, t] = np.asarray(inputs[f"b2_g{n}"][row])
    for pi, pr in enumerate(L3_PAIRS):
        for k, t in enumerate(pr):
            n, row, _ = TARGET[t]
            b3p[64 * k:64 * k + 64, pi] = np.asarray(inputs[f"b3_g{n}"][row])
    return dict(
        w1p=w1p.astype(bf16), w2p=w2p.astype(bf16), w3p=w3p.astype(bf16),
        b1p=b1p, b2p=b2p, b3p=b3p,
    )


def numpy_emulate(inputs, x):
    """Bit-layout-faithful numpy model of what the HW kernel computes (minus
    PSUM rounding): used to validate the plan / packing offline."""
    bf16 = ml_dtypes.bfloat16
    packed = pack_weights(inputs)
    # truncate-to-bf16 (high 16 bits of fp32), as the bitcast transpose sees
    xb = (np.ascontiguousarray(x, np.float32).view(np.uint32) >> 16).astype(
        np.uint16).view(bf16)                            # [Bn, 21, 64]
    Bn = x.shape[0]
    out = np.zeros((Bn, 21, 64), np.float32)
    for t in range(21):
        n, row, S = TARGET[t]
        psum1 = np.zeros((128, Bn), np.float32)
        for ci in range(n):
            col = CHUNK_COLS[(t, ci)]
            lhsT = packed["w1p"][:, col:col + 128].astype(np.float32)
            rhs = np.zeros((128, Bn), np.float32)
            rhs[1::2] = xb[:, S[ci]].astype(np.float32).T
            psum1 += lhsT.T @ rhs
        h1 = np.maximum(psum1 + packed["b1p"][:, t:t + 1], 0).astype(bf16)
        w2 = packed["w2p"][:, 128 * t:128 * (t + 1)].astype(np.float32)
        psum2 = w2.T @ h1.astype(np.float32)
        h2 = np.maximum(psum2 + packed["b2p"][:, t:t + 1], 0).astype(bf16)
        w3 = packed["w3p"][:, 64 * t:64 * (t + 1)].astype(np.float32)
        psum3 = w3.T @ h2.astype(np.float32)             # [64, Bn]
        pi, k = (t // 2, t % 2) if t < 20 else (10, 0)
        b3 = packed["b3p"][64 * k:64 * k + 64, pi]
        out[:, t] = (psum3 + b3[:, None]).astype(bf16).astype(np.float32).T
    return out


# ---------------------------------------------------------------------------
# Bass kernel
# ---------------------------------------------------------------------------

def build_bass_kernel():
    import concourse.bass as bass
    import concourse.tile as tile
    from concourse import bacc, mybir

    bf16 = mybir.dt.bfloat16
    f32 = mybir.dt.float32
    Relu = mybir.ActivationFunctionType.Relu
    Ident = mybir.ActivationFunctionType.Identity
    Alu = mybir.AluOpType

    nc = bacc.Bacc("TRN2", target_bir_lowering=False, debug=False,
                   num_devices=NCORES)
    x_dram = nc.dram_tensor("x", [BC, J, D], f32, kind="ExternalInput").ap()
    out_dram = nc.dram_tensor("out", [J * D, BC], bf16, kind="ExternalOutput").ap()
    w1_dram = nc.dram_tensor("w1p", [128, 128 * TOTAL_CHUNKS], bf16,
                             kind="ExternalInput").ap()
    w2_dram = nc.dram_tensor("w2p", [128, 128 * 21], bf16, kind="ExternalInput").ap()
    w3_dram = nc.dram_tensor("w3p", [128, 64 * 21], bf16, kind="ExternalInput").ap()
    b1_dram = nc.dram_tensor("b1p", [128, 21], f32, kind="ExternalInput").ap()
    b2_dram = nc.dram_tensor("b2p", [128, 21], f32, kind="ExternalInput").ap()
    b3_dram = nc.dram_tensor("b3p", [128, len(L3_PAIRS)], f32,
                             kind="ExternalInput").ap()

    # bf16 bitcast view of x: [BC, J, 128]; node j's block is contiguous,
    # odd columns are truncated-bf16 features, even columns +0.0
    x_bits = x_dram.bitcast(bf16)

    with tile.TileContext(nc) as tc:
        with (
            tc.tile_pool(name="wpool", bufs=1) as wpool,
            tc.tile_pool(name="xtp", bufs=2) as xtp,
            tc.tile_pool(name="actp", bufs=2) as actp,
            tc.tile_pool(name="h2p", bufs=2) as h2p,
            tc.tile_pool(name="stgp", bufs=2) as stgp,
            tc.tile_pool(name="ps1", bufs=2, space="PSUM") as ps1,
            tc.tile_pool(name="ps2", bufs=1, space="PSUM") as ps2,
            tc.tile_pool(name="ps3", bufs=2, space="PSUM") as ps3,
        ):
            w1s = wpool.tile([128, 128 * TOTAL_CHUNKS], bf16, name="w1s")
            w2s = wpool.tile([128, 128 * 21], bf16, name="w2s")
            w3s = wpool.tile([128, 64 * 21], bf16, name="w3s")
            b1s = wpool.tile([128, 21], f32, name="b1s")
            b2s = wpool.tile([128, 21], f32, name="b2s")
            b3s = wpool.tile([128, len(L3_PAIRS)], f32, name="b3s")
            nc.sync.dma_start(w1s[:], w1_dram)
            nc.sync.dma_start(w2s[:], w2_dram)
            nc.sync.dma_start(w3s[:], w3_dram)
            nc.sync.dma_start(b1s[:], b1_dram)
            nc.sync.dma_start(b2s[:], b2_dram)
            nc.sync.dma_start(b3s[:], b3_dram)

            # evac engine round-robin between the two PSUM readers
            evac_state = [0]

            def evac(dst, src, bias, relu):
                evac_state[0] ^= 1
                if evac_state[0]:
                    nc.scalar.activation(dst, src, Relu if relu else Ident,
                                         bias=bias, scale=1.0)
                else:
                    if relu:
                        nc.vector.tensor_scalar(dst, src, bias, 0.0,
                                                Alu.add, Alu.max)
                    else:
                        nc.vector.tensor_scalar(dst, src, bias, None, Alu.add)

            for it in range(NTILES):
                b0 = it * TILE
                # ---- per-node feature-major tiles via bitcast transpose ----
                # ONE xbar transpose for all 21 nodes: source [TILE, 21*128]
                # -> dest [128, 21, TILE] (3D out: dim 1 is the source column
                # group, i.e. the node).  Single instruction per tile keeps
                # the Sync sequencer off the critical path.
                xall = xtp.tile([128, J * TILE], bf16, tag="xall", name="xall")
                nc.sync.dma_start(
                    xall[:].rearrange("p (j b) -> p j b", b=TILE),
                    x_bits[b0:b0 + TILE].rearrange("b j f -> b (j f)"),
                    transpose=True)
                xN = [xall[:, TILE * j:TILE * (j + 1)] for j in range(J)]

                def mlp12(t):
                    n, _, S = TARGET[t]
                    psum1 = ps1.tile([128, TILE], f32, tag="psum1", name="psum1")
                    for h in range(TILE // 512):
                        for ci in range(n):
                            col = CHUNK_COLS[(t, ci)]
                            nc.tensor.matmul(
                                psum1[:, 512 * h:512 * (h + 1)],
                                w1s[:, col:col + 128],
                                xN[S[ci]][:, 512 * h:512 * (h + 1)],
                                start=(ci == 0), stop=(ci == n - 1))
                    h1 = actp.tile([128, TILE], bf16, tag="h1", name="h1")
                    evac(h1[:], psum1[:], b1s[:, t:t + 1], relu=True)

                    psum2 = ps2.tile([128, TILE], f32, tag="psum2", name="psum2")
                    for h in range(TILE // 512):
                        nc.tensor.matmul(
                            psum2[:, 512 * h:512 * (h + 1)],
                            w2s[:, 128 * t:128 * (t + 1)],
                            h1[:, 512 * h:512 * (h + 1)],
                            start=True, stop=True)
                    h2 = h2p.tile([128, TILE], bf16, tag=f"h2_{t % 4}",
                                  name=f"h2_{t % 4}")
                    evac(h2[:], psum2[:], b2s[:, t:t + 1], relu=True)
                    return h2

                # ---- fused L1/L2 then L3 per pair of targets ----
                for pi, pr in enumerate(L3_PAIRS):
                    h2t = [mlp12(t) for t in pr]
                    m = 64 * len(pr)
                    stg = stgp.tile([m, TILE], bf16, tag=f"stg{pi}",
                                    name=f"stg{pi}")
                    for h in range(TILE // 512):
                        psum3 = ps3.tile([m, 512], f32, tag="psum3", name="psum3")
                        for k, t in enumerate(pr):
                            nc.tensor.matmul(
                                psum3[64 * k:64 * (k + 1), :],
                                w3s[:, 64 * t:64 * (t + 1)],
                                h2t[k][:, 512 * h:512 * (h + 1)],
                                start=True, stop=True,
                                skip_group_check=True)
                        evac(stg[:, 512 * h:512 * (h + 1)], psum3[:],
                             b3s[0:m, pi:pi + 1], relu=False)
                    nc.gpsimd.dma_start(
                        out_dram[128 * pi:128 * pi + m, b0:b0 + TILE], stg[:])

    nc.compile()
    return nc


PACKED = None
_NC = None
LAST_RESULT = None


def prepare(inputs):
    """Build (once) the bass module and the per-core input maps."""
    global PACKED, _NC
    import sys
    if "/opt/trn_rl_repo" not in sys.path:
        sys.path.insert(0, "/opt/trn_rl_repo")
    x = np.ascontiguousarray(np.asarray(inputs["x"], np.float32))
    # zero the low 16 bits of every fp32 (truncate to bf16 precision) so the
    # even bf16 columns of the bitcast view are +0.0, never NaN/Inf garbage
    x = (x.view(np.uint32) & np.uint32(0xFFFF0000)).view(np.float32)
    PACKED = pack_weights(inputs)
    if _NC is None:
        _NC = build_bass_kernel()
    in_maps = []
    for core in range(NCORES):
        m = dict(PACKED)
        m["x"] = x[core * BC:(core + 1) * BC]
        in_maps.append(m)
    return _NC, in_maps


def kernel(**inputs):
    global LAST_RESULT
    nc, in_maps = prepare(inputs)
    from concourse.bass_utils import run_bass_kernel_spmd
    res = run_bass_kernel_spmd(nc, in_maps, core_ids=list(range(NCORES)))
    LAST_RESULT = res
    # per-core out: [21*64, BC] bf16, feature-major
    full = np.concatenate([r["out"] for r in res.results], 1)  # [1344, B]
    return np.ascontiguousarray(
        full.reshape(J, D, B).transpose(2, 0, 1)).astype(np.float32)
